# revision 13
# baseline (speedup 1.0000x reference)
"""Trainium2 Bass kernel for CausalSelfAttention (PentaNet-quantized weights).

Reference computation (B=2, T=2048, C=1024, H=16 heads, D=64):
    qkv = x @ quant(w_attn).T ; split q,k,v ; causal softmax attention ;
    out = y @ quant(w_proj).T

Sharding: 8 cores = 2 (batch) x 4 (head groups of 4 heads).  Each core
computes its batch element's attention for its 4 heads plus the partial
output projection over its 256 input channels; the host sums the 4
partials per batch (the w_proj contraction is split across head groups).

Device layout avoids all on-chip transposes:
  - host supplies xT = x[b].T  [C, T]
  - qT,kT computed as [o, t] (weights stationary), v as [t, o]
  - scores computed transposed: ST[j, i] = k_j . q_i  (j = key pos)
  - P = exp(ST/8) with causal masking (block-skip + triangular mask)
  - OT[d, i] = sum_j V[j, d] P[j, i] accumulated in PSUM; an extra
    ones-column in V yields the softmax denominator as OT row 64
  - OT normalized is exactly the lhsT the projection needs.
All matmuls run in bf16 (fp32 PSUM accumulation): bf16 gets full-rate
streaming plus pipelined fast-weight-load, where fp32/fp32r matmuls
self-load weights serially (~100ns extra per matmul, ~60us/core here);
measured end-to-end rel err vs the fp32 reference is ~5e-3.
"""

import os
import sys

sys.path.insert(0, "/opt/trn_rl_repo")

import numpy as np
import ml_dtypes

import jax

try:
    jax.config.update("jax_compilation_cache_dir", "/root/.cache/jax_bass_neff")
except Exception:
    pass

import concourse.bass as bass
import concourse.tile as tile
from concourse import bacc, mybir
from concourse.bass_utils import run_bass_kernel_spmd

F32 = mybir.dt.float32
F32R = mybir.dt.float32r
BF16 = mybir.dt.bfloat16

B, T, C = 2, 2048, 1024
H, D = 16, 64
HL = 4                    # heads per core
OL = HL * D               # 256 local output channels
KT = C // 128             # 8 k-tiles over C
TT = T // 128             # 16 t-tiles
NCH = T // 512            # 4 i-chunks of 512
SCALE = 1.0 / 8.0         # 1/sqrt(D)
VW = 128                  # V block width: col 0 ones, 1-63 zero, 64-127 data


def r(ap):
    return ap


def build_body(ctx, tc, xT, wq, wk, wv, wp, tri, ztri, tri2, onesd, out):
    nc = tc.nc

    consts = ctx.enter_context(tc.tile_pool(name="consts", bufs=1))
    acts = ctx.enter_context(tc.tile_pool(name="acts", bufs=1))
    pp = ctx.enter_context(tc.tile_pool(name="pp", bufs=4))
    rcp = ctx.enter_context(tc.tile_pool(name="rcp", bufs=2))
    bbp = ctx.enter_context(tc.tile_pool(name="bbp", bufs=2))
    obp = ctx.enter_context(tc.tile_pool(name="obp", bufs=3))
    ps_mm = ctx.enter_context(tc.tile_pool(name="ps_mm", bufs=2, space="PSUM"))
    ps_pj = ctx.enter_context(tc.tile_pool(name="ps_pj", bufs=2, space="PSUM"))
    ps_ot = ctx.enter_context(tc.tile_pool(name="ps_ot", bufs=2, space="PSUM"))

    # ---- load inputs to SBUF ----
    # weights first, then xT by column chunk so qkv matmuls start early
    wq_sb = consts.tile([128, KT * OL], BF16)
    wk_sb = consts.tile([128, KT * OL], BF16)
    wv_sb = consts.tile([128, KT * OL], BF16)
    xT_sb = consts.tile([128, KT * T], BF16)

    def load_w(w_sb, w_d):
        # one DMA: DRAM [KT*128, OL] -> SBUF [128, KT, OL]
        nc.sync.dma_start(
            w_sb[:].rearrange("p (k o) -> p k o", k=KT),
            w_d.rearrange("(k p) o -> p k o", k=KT))

    def load_x(n):
        # one strided DMA per column chunk: DRAM [KT*128, 512] -> SBUF [128, KT, 512]
        nc.sync.dma_start(
            xT_sb[:].rearrange("p (k t) -> p k t", k=KT)[:, :, n * 512:(n + 1) * 512],
            xT.rearrange("(k p) t -> p k t", k=KT)[:, :, n * 512:(n + 1) * 512])

    load_w(wq_sb, wq)
    load_w(wv_sb, wv)
    load_x(0)
    load_w(wk_sb, wk)
    for n in range(1, NCH):
        load_x(n)
    tri_sb = consts.tile([128, 128], BF16)
    nc.sync.dma_start(tri_sb[:], tri[:, 0:128])
    ztri_sb = consts.tile([128, 256], BF16)
    nc.sync.dma_start(ztri_sb[:], ztri[:, :])
    tri_sb2 = consts.tile([128, 256], BF16)
    nc.sync.dma_start(tri_sb2[:], tri2[:, :])
    wp_sb = consts.tile([128, 2 * C], BF16)
    nc.sync.dma_start(
        wp_sb[:].rearrange("p (k o) -> p k o", k=2),
        wp.rearrange("(k p) o -> p k o", k=2))

    q_sb = acts.tile([128, 2 * T], BF16)
    k_sb = acts.tile([128, 2 * T], BF16)
    v_sb = acts.tile([128, TT * HL * VW], BF16)
    yt_sb = acts.tile([128, 2 * T], BF16)
    # cols 0-31 of every [t-tile, head] V block: col 0 ones (denominator on
    # partition 0 of ps_o: reciprocal_approx_fast misreads base_partition=64,
    # and engine APs must start at a 32-aligned partition), cols 1-31 zero
    v_ones = v_sb[:].rearrange("p (g c) -> p g c", c=VW)[:, :, 0:64]
    nc.sync.dma_start(v_ones, onesd.rearrange("p (g c) -> p g c", c=64))

    # ---- emission units ----
    # qk_unit / v_unit / proj_unit: one PSUM accumulation group each (PE filler
    # work).  attn blocks: the ST->exp->OT chain that keeps ACT busy.
    def qk_unit(n, which, m):
        w_sb, dst = (wq_sb, q_sb) if which == 0 else (wk_sb, k_sb)
        ps = ps_pj.tile([128, 512], F32, tag="pj")
        for k in range(KT):
            nc.tensor.matmul(
                ps[:],
                r(w_sb[:, k * OL + m * 128: k * OL + (m + 1) * 128]),
                r(xT_sb[:, k * T + n * 512: k * T + (n + 1) * 512]),
                start=(k == 0), stop=(k == KT - 1),
            )
        nc.vector.tensor_copy(dst[:, m * T + n * 512: m * T + (n + 1) * 512], ps[:])

    def v_unit(t):
        ps = ps_pj.tile([128, OL], F32, tag="pj")
        for k in range(KT):
            nc.tensor.matmul(
                ps[:],
                r(xT_sb[:, k * T + t * 128: k * T + (t + 1) * 128]),
                r(wv_sb[:, k * OL:(k + 1) * OL]),
                start=(k == 0), stop=(k == KT - 1),
            )
        dst = v_sb[:, t * HL * VW: (t + 1) * HL * VW]
        dst = dst.rearrange("p (h c) -> p h c", h=HL)[:, :, 64:64 + D]
        nc.vector.tensor_copy(dst, ps[:].rearrange("p (h c) -> p h c", h=HL))

    def proj_unit(t, n2):
        # n2 == 0 computes cols [0:512] into ob; n2 == 1 computes [512:1024]
        # and issues the single merged store for the whole t-tile row block.
        ps = ps_pj.tile([128, 512], F32, tag="pj")
        for kk in range(2):
            nc.tensor.matmul(
                ps[:],
                r(yt_sb[:, kk * T + t * 128: kk * T + (t + 1) * 128]),
                r(wp_sb[:, kk * C + n2 * 512: kk * C + (n2 + 1) * 512]),
                start=(kk == 0), stop=(kk == 1),
            )
        ob = _ob_state.get(t)
        if ob is None:
            ob = obp.tile([128, 1024], BF16, tag="ob", name=f"ob_{t}")
            _ob_state[t] = ob
        if n2 == 0:
            nc.vector.tensor_copy(ob[:, 0:512], ps[:])
        else:
            nc.vector.tensor_copy(ob[:, 512:1024], ps[:])
            nc.sync.dma_start(out[t * 128:(t + 1) * 128, :], ob[:])
            del _ob_state[t]

    _ob_state = {}

    def qkv_units(n):
        return ([(lambda n=n, w=w, m=m: qk_unit(n, w, m)) for w in range(2) for m in range(2)]
                + [(lambda t=t: v_unit(t)) for t in range(4 * n, 4 * n + 4)])

    def proj_units(ic):
        return [(lambda t=t, n2=n2: proj_unit(t, n2))
                for t in range(4 * ic, 4 * ic + 4) for n2 in range(2)]

    # attention chunk as a list of work-item callables; the chunk's ps_o
    # tiles live across its items.  Full blocks (tj < 4*ic) are processed in
    # pairs sharing one [128,1024] PSUM tile and one exp; diagonal blocks
    # are merged in pairs too (d=3 widened to 256 cols, extra cols masked).
    def attn_blocks(ic):
        # Software-pipelined: per head pair, emit scores/exp(k+1) before the
        # AV matmuls of block k, so the PE streams the next score pair while
        # ACT computes exp(k).  Score matmuls for the two heads of a pair are
        # back-to-back: K=64 contraction -> disjoint PE row groups run
        # concurrently (~2x score throughput).
        items = []
        for hp in range(2):
            mo = hp * T
            state = {}

            def open_ps(hp=hp, ic=ic, state=state):
                for s in range(2):
                    state[s] = ps_ot.tile([VW, 512], F32, tag="ot",
                                          name=f"ps_o_{ic}_{hp}_{s}")

            def ot_mm(s, tj, rhs, cs, hp=hp, ic=ic, state=state):
                h = 2 * hp + s
                vh = v_sb[:, (tj * HL + h) * VW:(tj * HL + h + 1) * VW]
                nc.tensor.matmul(
                    state[s][:, cs:512],
                    r(vh), r(rhs),
                    start=(tj == 0), stop=(tj == 4 * ic + 3),
                    skip_group_check=True,
                )

            def full_sc(tj, hp=hp, mo=mo, ic=ic, state=state):
                ps_s = ps_mm.tile([128, 1024], F32, tag="mm",
                                  name=f"ps_s_{ic}_{hp}_{tj}")
                for s in range(2):
                    nc.tensor.matmul(
                        ps_s[:, s * 512:(s + 1) * 512],
                        r(k_sb[64 * s:64 * s + 64, mo + tj * 128: mo + (tj + 1) * 128]),
                        r(q_sb[64 * s:64 * s + 64, mo + ic * 512: mo + (ic + 1) * 512]),
                        start=True, stop=True,
                        skip_group_check=True,
                    )
                p_t = pp.tile([128, 1024], BF16, tag="p", name=f"p_t_{ic}_{hp}_{tj}")
                nc.scalar.activation(p_t[:], ps_s[:], mybir.ActivationFunctionType.Exp,
                                     scale=SCALE)
                state[("p", tj)] = p_t

            def full_av(tj, hp=hp, ic=ic, state=state, open_ps=open_ps, ot_mm=ot_mm):
                if tj == 0:
                    open_ps()
                p_t = state.pop(("p", tj))
                for s in range(2):
                    ot_mm(s, tj, p_t[:, s * 512:(s + 1) * 512], 0)

            def diag_sc(da, hp=hp, mo=mo, ic=ic, state=state):
                if da == 0:
                    widths, css = (512, 384), (0, 128)
                else:
                    widths, css = (256, 128), (256, 384)
                tot_w = widths[0] + widths[1]
                ps_d = [ps_mm.tile([128, tot_w], F32, tag="mm",
                                   name=f"ps_d_{ic}_{hp}_{da}_{s}")
                        for s in range(2)]
                for j in range(2):
                    off = 0 if j == 0 else widths[0]
                    tj = 4 * ic + da + j
                    for s in range(2):
                        nc.tensor.matmul(
                            ps_d[s][:, off:off + widths[j]],
                            r(k_sb[64 * s:64 * s + 64, mo + tj * 128: mo + (tj + 1) * 128]),
                            r(q_sb[64 * s:64 * s + 64, mo + ic * 512 + css[j]: mo + (ic + 1) * 512]),
                            start=True, stop=True,
                            skip_group_check=True,
                        )
                for s in range(2):
                    p_t = pp.tile([128, tot_w], BF16, tag="p",
                                  name=f"p_d_{ic}_{hp}_{da}_{s}")
                    nc.scalar.activation(p_t[:], ps_d[s][:],
                                         mybir.ActivationFunctionType.Exp,
                                         scale=SCALE)
                    if da == 0:
                        nc.vector.tensor_mul(p_t[:, 0:128], p_t[:, 0:128], tri_sb[:])
                        nc.vector.tensor_mul(p_t[:, 512:640], p_t[:, 512:640], tri_sb[:])
                    else:
                        nc.vector.tensor_mul(p_t[:, 0:256], p_t[:, 0:256], tri_sb2[:])
                        nc.vector.tensor_mul(p_t[:, 256:384], p_t[:, 256:384], tri_sb[:])
                    state[("pd", da, s)] = p_t

            def diag_av(da, hp=hp, mo=mo, ic=ic, state=state, open_ps=open_ps, ot_mm=ot_mm):
                if 4 * ic + da == 0:
                    open_ps()
                if da == 0:
                    widths, css = (512, 384), (0, 128)
                else:
                    widths, css = (256, 128), (256, 384)
                for s in range(2):
                    p_t = state.pop(("pd", da, s))
                    off = 0
                    for j in range(2):
                        ot_mm(s, 4 * ic + da + j, p_t[:, off:off + widths[j]], css[j])
                        off += widths[j]
                if da == 2:
                    # evict ps_o data rows to SBUF right away (releases the
                    # PSUM banks ~1.5us earlier for the next pair's AV), then
                    # normalize from the SBUF copy.
                    ys = bbp.tile([128, 512], BF16, tag="ys")
                    for s in range(2):
                        ps_o = state[s]
                        rc = rcp.tile([1, 512], F32, tag="rc")
                        nc.vector.reciprocal_approx_fast(rc[:], ps_o[0:1, :])
                        nc.vector.tensor_copy(ys[64 * s:64 * s + 64, :],
                                              ps_o[64:64 + D, :])
                        bb = bbp.tile([64, 512], F32, tag="bb")
                        nc.gpsimd.partition_broadcast(bb[:], rc[:])
                        nc.vector.tensor_mul(
                            yt_sb[64 * s:64 * s + 64, mo + ic * 512: mo + (ic + 1) * 512],
                            ys[64 * s:64 * s + 64, :], bb[:],
                        )

            sc_items = [(lambda tj=tj, f=full_sc: f(tj)) for tj in range(4 * ic)]
            sc_items += [(lambda da=da, f=diag_sc: f(da)) for da in (0, 2)]
            av_items = [(lambda tj=tj, f=full_av: f(tj)) for tj in range(4 * ic)]
            av_items += [(lambda da=da, f=diag_av: f(da)) for da in (0, 2)]

            items.append(sc_items[0])
            for i in range(1, len(sc_items)):
                items.append(sc_items[i])
                items.append(av_items[i - 1])
            items.append(av_items[-1])
        return items

    def emit_interleaved(blocks, fillers):
        """Emit attention blocks with filler units spread evenly between."""
        nf = len(fillers)
        nb = len(blocks)
        fi = 0
        for i, blk in enumerate(blocks):
            blk()
            want = (i + 1) * nf // nb
            while fi < want:
                fillers[fi]()
                fi += 1
        while fi < nf:
            fillers[fi]()
            fi += 1

    # schedule: qkv(0) first; attention chunk ic interleaves qkv(ic+1) and
    # proj(ic-1); proj(3) trails.
    for u in qkv_units(0):
        u()
    for ic in range(NCH):
        fill = []
        if ic + 1 < NCH:
            fill += qkv_units(ic + 1)
        if ic - 1 >= 0:
            fill += proj_units(ic - 1)
        emit_interleaved(attn_blocks(ic), fill)
    for u in proj_units(NCH - 1):
        u()


def build_program(reps=1):
    from contextlib import ExitStack

    nc = bacc.Bacc("TRN2", target_bir_lowering=False, debug=False)
    xT = nc.dram_tensor("xT", [C, T], BF16, kind="ExternalInput").ap()
    wq = nc.dram_tensor("wq", [C, OL], BF16, kind="ExternalInput").ap()
    wk = nc.dram_tensor("wk", [C, OL], BF16, kind="ExternalInput").ap()
    wv = nc.dram_tensor("wv", [C, OL], BF16, kind="ExternalInput").ap()
    wp = nc.dram_tensor("wp", [OL, C], BF16, kind="ExternalInput").ap()
    tri = nc.dram_tensor("tri", [128, 128], BF16, kind="ExternalInput").ap()
    ztri = nc.dram_tensor("ztri", [128, 256], BF16, kind="ExternalInput").ap()
    tri2 = nc.dram_tensor("tri2", [128, 256], BF16, kind="ExternalInput").ap()
    onesd = nc.dram_tensor("onesd", [128, TT * HL * 64], BF16, kind="ExternalInput").ap()
    out = nc.dram_tensor("out", [T, C], BF16, kind="ExternalOutput").ap()

    with tile.TileContext(nc) as tc:
        for _ in range(reps):
            with ExitStack() as ctx:
                build_body(ctx, tc, xT, wq, wk, wv, wp, tri, ztri, tri2, onesd, out)
    nc.compile()
    return nc


def quant_weight_np(w):
    scale = max(np.mean(np.abs(w), dtype=np.float32), np.float32(1e-8))
    return (np.clip(np.round(w / scale), -2.0, 2.0) * scale).astype(np.float32)


def make_in_maps(x, w_attn, w_proj):
    wq_f = quant_weight_np(w_attn)
    wp_f = quant_weight_np(w_proj)
    onesd_v = np.zeros((128, TT * HL, 64), dtype=ml_dtypes.bfloat16)
    onesd_v[:, :, 0] = 1
    onesd_v = onesd_v.reshape(128, TT * HL * 64)
    tri = np.triu(np.ones((128, 128), dtype=np.float32))
    ztri = np.concatenate([np.zeros((128, 128), dtype=np.float32), tri], axis=1)
    tri2 = np.concatenate([tri, np.ones((128, 128), dtype=np.float32)], axis=1)
    in_maps = []
    for core in range(8):
        b, g = divmod(core, 4)
        sl = slice(g * OL, (g + 1) * OL)
        in_maps.append({
            "xT": np.ascontiguousarray(x[b].T).astype(ml_dtypes.bfloat16),
            "wq": np.ascontiguousarray(wq_f[0 * C:1 * C][sl].T).astype(ml_dtypes.bfloat16),
            "wk": np.ascontiguousarray(wq_f[1 * C:2 * C][sl].T).astype(ml_dtypes.bfloat16),
            "wv": np.ascontiguousarray(wq_f[2 * C:3 * C][sl].T).astype(ml_dtypes.bfloat16),
            "wp": np.ascontiguousarray(wp_f[:, sl].T).astype(ml_dtypes.bfloat16),
            "tri": tri.astype(ml_dtypes.bfloat16),
            "ztri": ztri.astype(ml_dtypes.bfloat16),
            "tri2": tri2.astype(ml_dtypes.bfloat16),
            "onesd": onesd_v,
        })
    return in_maps


_CACHED_NC = None


def kernel(x, w_attn, w_proj):
    global _CACHED_NC
    if _CACHED_NC is None:
        _CACHED_NC = build_program()
    in_maps = make_in_maps(np.asarray(x, dtype=np.float32),
                           np.asarray(w_attn, dtype=np.float32),
                           np.asarray(w_proj, dtype=np.float32))
    res = run_bass_kernel_spmd(_CACHED_NC, in_maps, list(range(8)))
    out = np.zeros((B, T, C), dtype=np.float32)
    for core in range(8):
        b = core // 4
        out[b] += res.results[core]["out"]
    return out



# revision 18
# speedup vs baseline: 4.6587x; 4.6587x over previous
"""Trainium2 Bass kernel for CausalSelfAttention (PentaNet-quantized weights).

Reference computation (B=2, T=2048, C=1024, H=16 heads, D=64):
    qkv = x @ quant(w_attn).T ; split q,k,v ; causal softmax attention ;
    out = y @ quant(w_proj).T

Sharding: 8 cores = 2 (batch) x 4 (head groups of 4 heads).  Each core
computes its batch element's attention for its 4 heads plus the partial
output projection over its 256 input channels; the host sums the 4
partials per batch (the w_proj contraction is split across head groups).

Device layout avoids all on-chip transposes:
  - host supplies xT = x[b].T  [C, T]
  - qT,kT computed as [o, t] (weights stationary), v as [t, o]
  - scores computed transposed: ST[j, i] = k_j . q_i  (j = key pos)
  - score matmuls for the two heads of a pair (partition halves 0-63 /
    64-127, K=64 contraction) are emitted back-to-back so they run
    concurrently on disjoint PE row groups (~2x score throughput)
  - P = exp(ST/8) with causal masking (block-skip + triangular mask)
  - OT[d, i] = sum_j V[j, d] P[j, i] accumulated in PSUM; V blocks are
    128 wide: col 0 ones (softmax denominator lands on partition 0 of
    the accumulator), cols 1-63 zero, cols 64-127 the actual V
  - the attention inner loop is software-pipelined: scores/exp of block
    k+1 are emitted before the AV matmuls of block k
  - normalized OT is exactly the lhsT the projection needs.
All matmuls run in bf16 (fp32 PSUM accumulation).  Input/activation
SBUF buffers are double-buffered across reps so the next rep's input
DMA overlaps this rep's compute.
"""

import os
import sys

sys.path.insert(0, "/opt/trn_rl_repo")

import numpy as np
import ml_dtypes

import jax

try:
    jax.config.update("jax_compilation_cache_dir", "/root/.cache/jax_bass_neff")
except Exception:
    pass

import concourse.bass as bass
import concourse.tile as tile
from concourse import bacc, mybir
from concourse.bass_utils import run_bass_kernel_spmd

F32 = mybir.dt.float32
F32R = mybir.dt.float32r
BF16 = mybir.dt.bfloat16

B, T, C = 2, 2048, 1024
H, D = 16, 64
HL = 4                    # heads per core
OL = HL * D               # 256 local output channels
KT = C // 128             # 8 k-tiles over C
TT = T // 128             # 16 t-tiles
NCH = T // 512            # 4 i-chunks of 512
SCALE = 1.0 / 8.0         # 1/sqrt(D)
VW = 128                  # V block width: col 0 ones, 1-63 zero, 64-127 data


def r(ap):
    return ap


def build_body(tc, pools, rep, shared, carry_in,
               xT, wq, wk, wv, wp, tri, tri2, onesd, out):
    nc = tc.nc
    inp2 = pools["inp2"]
    cst1 = pools["cst1"]
    act2 = pools["act2"]
    pp = pools["pp"]
    rcp = pools["rcp"]
    bbp = pools["bbp"]
    obp = pools["obp"]
    ps_mm = pools["ps_mm"]
    ps_pj = pools["ps_pj"]
    ps_ot = pools["ps_ot"]

    # ---- load inputs to SBUF ----
    # weights first, then xT by column chunk so qkv matmuls start early.
    # inp2/act2 are double-buffered across reps: these DMAs have no WAR
    # hazard on the previous rep and run under its compute.
    wq_sb = inp2.tile([128, KT * OL], BF16, tag="wq", name="wq_sb")
    wk_sb = inp2.tile([128, KT * OL], BF16, tag="wk", name="wk_sb")
    wv_sb = inp2.tile([128, KT * OL], BF16, tag="wv", name="wv_sb")
    xT_sb = inp2.tile([128, KT * T], BF16, tag="xT", name="xT_sb")
    wp_sb = inp2.tile([128, 2 * C], BF16, tag="wp", name="wp_sb")

    def load_w(w_sb, w_d):
        # one DMA: DRAM [KT*128, OL] -> SBUF [128, KT, OL]
        nc.sync.dma_start(
            w_sb[:].rearrange("p (k o) -> p k o", k=KT),
            w_d.rearrange("(k p) o -> p k o", k=KT))

    def load_x(n):
        # one strided DMA per column chunk: DRAM [KT*128, 512] -> SBUF [128, KT, 512]
        nc.sync.dma_start(
            xT_sb[:].rearrange("p (k t) -> p k t", k=KT)[:, :, n * 512:(n + 1) * 512],
            xT.rearrange("(k p) t -> p k t", k=KT)[:, :, n * 512:(n + 1) * 512])

    q_sb = act2.tile([128, 2 * T], BF16, tag="q", name="q_sb")
    k_sb = act2.tile([128, 2 * T], BF16, tag="k", name="k_sb")
    v_sb = act2.tile([128, TT * HL * VW], BF16, tag="v", name="v_sb")
    yt_sb = act2.tile([128, 2 * T], BF16, tag="yt", name="yt_sb")

    load_w(wq_sb, wq)
    load_w(wv_sb, wv)
    load_x(0)
    load_w(wk_sb, wk)
    if rep < 2:
        # cols 0-63 of every [t-tile, head] V block: col 0 ones (softmax
        # denominator on partition 0 of ps_o: reciprocal_approx_fast and
        # partition_broadcast only operate at base_partition 0), cols 1-63
        # zero.  The compute only writes cols 64-127, so each of the two
        # rotating v buffers keeps this pattern after its first load.
        v_ones = v_sb[:].rearrange("p (g c) -> p g c", c=VW)[:, :, 0:64]
        nc.sync.dma_start(v_ones, onesd.rearrange("p (g c) -> p g c", c=64))
    for n in range(1, NCH):
        load_x(n)
    if rep == 0:
        shared["tri_sb"] = cst1.tile([128, 128], BF16, tag="tri", name="tri_sb")
        nc.sync.dma_start(shared["tri_sb"][:], tri[:, 0:128])
        shared["tri_sb2"] = cst1.tile([128, 256], BF16, tag="tri2", name="tri_sb2")
        nc.sync.dma_start(shared["tri_sb2"][:], tri2[:, :])
    tri_sb = shared["tri_sb"]
    tri_sb2 = shared["tri_sb2"]
    nc.sync.dma_start(
        wp_sb[:].rearrange("p (k o) -> p k o", k=2),
        wp.rearrange("(k p) o -> p k o", k=2))

    # ---- emission units ----
    # qk_unit / v_unit / proj_unit: one PSUM accumulation group each (PE filler
    # work).  attn blocks: the ST->exp->OT chain that keeps ACT busy.
    def qk_unit(n, which, m):
        w_sb, dst = (wq_sb, q_sb) if which == 0 else (wk_sb, k_sb)
        ps = ps_pj.tile([128, 512], F32, tag="pj", name="ps_qk")
        for k in range(KT):
            nc.tensor.matmul(
                ps[:],
                r(w_sb[:, k * OL + m * 128: k * OL + (m + 1) * 128]),
                r(xT_sb[:, k * T + n * 512: k * T + (n + 1) * 512]),
                start=(k == 0), stop=(k == KT - 1),
            )
        nc.vector.tensor_copy(dst[:, m * T + n * 512: m * T + (n + 1) * 512], ps[:])

    def v_unit(t):
        ps = ps_pj.tile([128, OL], F32, tag="pj", name="ps_v")
        for k in range(KT):
            nc.tensor.matmul(
                ps[:],
                r(xT_sb[:, k * T + t * 128: k * T + (t + 1) * 128]),
                r(wv_sb[:, k * OL:(k + 1) * OL]),
                start=(k == 0), stop=(k == KT - 1),
            )
        dst = v_sb[:, t * HL * VW: (t + 1) * HL * VW]
        dst = dst.rearrange("p (h c) -> p h c", h=HL)[:, :, 64:64 + D]
        nc.vector.tensor_copy(dst, ps[:].rearrange("p (h c) -> p h c", h=HL))

    def proj_unit(t, n2):
        # n2 == 0 computes cols [0:512] into ob; n2 == 1 computes [512:1024]
        # and issues the single merged store for the whole t-tile row block.
        ps = ps_pj.tile([128, 512], F32, tag="pj", name="ps_pr")
        for kk in range(2):
            nc.tensor.matmul(
                ps[:],
                r(yt_sb[:, kk * T + t * 128: kk * T + (t + 1) * 128]),
                r(wp_sb[:, kk * C + n2 * 512: kk * C + (n2 + 1) * 512]),
                start=(kk == 0), stop=(kk == 1),
            )
        ob = _ob_state.get(t)
        if ob is None:
            ob = obp.tile([128, 1024], BF16, tag="ob", name=f"ob_{t}")
            _ob_state[t] = ob
        if n2 == 0:
            nc.vector.tensor_copy(ob[:, 0:512], ps[:])
        else:
            nc.scalar.copy(ob[:, 512:1024], ps[:])
            nc.sync.dma_start(out[t * 128:(t + 1) * 128, :], ob[:])
            del _ob_state[t]

    _ob_state = {}

    def qkv_units(n):
        return ([(lambda n=n, w=w, m=m: qk_unit(n, w, m)) for w in range(2) for m in range(2)]
                + [(lambda t=t: v_unit(t)) for t in range(4 * n, 4 * n + 4)])

    def proj_units(ic):
        return [(lambda t=t, n2=n2: proj_unit(t, n2))
                for t in range(4 * ic, 4 * ic + 4) for n2 in range(2)]

    # attention chunk as a list of work-item callables; the chunk's ps_o
    # tiles live across its items.
    def attn_blocks(ic):
        # Software-pipelined: per head pair, emit scores/exp(k+1) before the
        # AV matmuls of block k, so the PE streams the next score pair while
        # ACT computes exp(k).  Score matmuls for the two heads of a pair are
        # back-to-back: K=64 contraction -> disjoint PE row groups run
        # concurrently (~2x score throughput).
        items = []
        for hp in range(2):
            mo = hp * T
            state = {}

            def open_ps(hp=hp, ic=ic, state=state):
                for s in range(2):
                    state[s] = ps_ot.tile([VW, 512], F32, tag="ot",
                                          name=f"ps_o_{ic}_{hp}_{s}")

            def ot_mm(s, tj, rhs, cs, hp=hp, ic=ic, state=state):
                h = 2 * hp + s
                vh = v_sb[:, (tj * HL + h) * VW:(tj * HL + h + 1) * VW]
                nc.tensor.matmul(
                    state[s][:, cs:512],
                    r(vh), r(rhs),
                    start=(tj == 0), stop=(tj == 4 * ic + 3),
                    skip_group_check=True,
                )

            def full_sc(tj, hp=hp, mo=mo, ic=ic, state=state):
                ps_s = ps_mm.tile([128, 1024], F32, tag="mm",
                                  name=f"ps_s_{ic}_{hp}_{tj}")
                for s in range(2):
                    nc.tensor.matmul(
                        ps_s[:, s * 512:(s + 1) * 512],
                        r(k_sb[64 * s:64 * s + 64, mo + tj * 128: mo + (tj + 1) * 128]),
                        r(q_sb[64 * s:64 * s + 64, mo + ic * 512: mo + (ic + 1) * 512]),
                        start=True, stop=True,
                        skip_group_check=True,
                    )
                p_t = pp.tile([128, 1024], BF16, tag="p", name=f"p_t_{ic}_{hp}_{tj}")
                nc.scalar.activation(p_t[:], ps_s[:], mybir.ActivationFunctionType.Exp,
                                     scale=SCALE)
                state[("p", tj)] = p_t

            def full_av(tj, hp=hp, ic=ic, state=state, open_ps=open_ps, ot_mm=ot_mm):
                if tj == 0:
                    open_ps()
                p_t = state.pop(("p", tj))
                for s in range(2):
                    ot_mm(s, tj, p_t[:, s * 512:(s + 1) * 512], 0)

            def diag_sc(da, hp=hp, mo=mo, ic=ic, state=state):
                if da == 0:
                    widths, css = (512, 384), (0, 128)
                else:
                    widths, css = (256, 128), (256, 384)
                tot_w = widths[0] + widths[1]
                ps_d = [ps_mm.tile([128, tot_w], F32, tag="mm",
                                   name=f"ps_d_{ic}_{hp}_{da}_{s}")
                        for s in range(2)]
                for j in range(2):
                    off = 0 if j == 0 else widths[0]
                    tj = 4 * ic + da + j
                    for s in range(2):
                        nc.tensor.matmul(
                            ps_d[s][:, off:off + widths[j]],
                            r(k_sb[64 * s:64 * s + 64, mo + tj * 128: mo + (tj + 1) * 128]),
                            r(q_sb[64 * s:64 * s + 64, mo + ic * 512 + css[j]: mo + (ic + 1) * 512]),
                            start=True, stop=True,
                            skip_group_check=True,
                        )
                for s in range(2):
                    p_t = pp.tile([128, tot_w], BF16, tag="p",
                                  name=f"p_d_{ic}_{hp}_{da}_{s}")
                    nc.scalar.activation(p_t[:], ps_d[s][:],
                                         mybir.ActivationFunctionType.Exp,
                                         scale=SCALE)
                    if da == 0:
                        nc.vector.tensor_mul(p_t[:, 0:128], p_t[:, 0:128], tri_sb[:])
                        nc.vector.tensor_mul(p_t[:, 512:640], p_t[:, 512:640], tri_sb[:])
                    else:
                        nc.vector.tensor_mul(p_t[:, 0:256], p_t[:, 0:256], tri_sb2[:])
                        nc.vector.tensor_mul(p_t[:, 256:384], p_t[:, 256:384], tri_sb[:])
                    state[("pd", da, s)] = p_t

            def diag_av(da, hp=hp, mo=mo, ic=ic, state=state, open_ps=open_ps, ot_mm=ot_mm):
                if 4 * ic + da == 0:
                    open_ps()
                if da == 0:
                    widths, css = (512, 384), (0, 128)
                else:
                    widths, css = (256, 128), (256, 384)
                for s in range(2):
                    p_t = state.pop(("pd", da, s))
                    off = 0
                    for j in range(2):
                        ot_mm(s, 4 * ic + da + j, p_t[:, off:off + widths[j]], css[j])
                        off += widths[j]
                if da == 2:
                    # evict ps_o data rows to SBUF right away (releases the
                    # PSUM banks ~1.5us earlier for the next pair's AV), then
                    # normalize from the SBUF copy.
                    for s in range(2):
                        ps_o = state[s]
                        rc = rcp.tile([1, 512], F32, tag="rc", name="rc")
                        nc.vector.reciprocal_approx_fast(rc[:], ps_o[0:1, :])
                        ys = bbp.tile([64, 512], BF16, tag="ys", name="ys")
                        nc.scalar.copy(ys[:], ps_o[64:64 + D, :])
                        bb = bbp.tile([64, 512], F32, tag="bb", name="bb")
                        nc.gpsimd.partition_broadcast(bb[:], rc[:])
                        nc.vector.tensor_mul(
                            yt_sb[64 * s:64 * s + 64, mo + ic * 512: mo + (ic + 1) * 512],
                            ys[:], bb[:],
                        )

            sc_items = [(lambda tj=tj, f=full_sc: f(tj)) for tj in range(4 * ic)]
            sc_items += [(lambda da=da, f=diag_sc: f(da)) for da in (0, 2)]
            av_items = [(lambda tj=tj, f=full_av: f(tj)) for tj in range(4 * ic)]
            av_items += [(lambda da=da, f=diag_av: f(da)) for da in (0, 2)]

            OFF = 2
            for i in range(len(sc_items)):
                items.append(sc_items[i])
                if i - OFF >= 0:
                    items.append(av_items[i - OFF])
            for j in range(max(0, len(av_items) - OFF), len(av_items)):
                items.append(av_items[j])
        return items

    def emit_interleaved(blocks, fillers):
        """Emit attention blocks with filler units spread evenly between."""
        nf = len(fillers)
        nb = len(blocks)
        fi = 0
        for i, blk in enumerate(blocks):
            blk()
            want = (i + 1) * nf // nb
            while fi < want:
                fillers[fi]()
                fi += 1
        while fi < nf:
            fillers[fi]()
            fi += 1

    # schedule: qkv(0) first; attention chunk ic interleaves qkv(ic+1) and
    # proj(ic-1).  proj(3) is NOT emitted here: it is returned so the next
    # rep can interleave it into its first chunk (it only depends on this
    # rep's yt buffer), keeping the PE queue free of the end-of-rep
    # normalize->proj dependency stall.
    for u in qkv_units(0):
        u()
    for ic in range(NCH):
        fill = []
        if ic == 0:
            fill += carry_in
        if ic + 1 < NCH:
            fill += qkv_units(ic + 1)
        if ic - 1 >= 0:
            fill += proj_units(ic - 1)
        emit_interleaved(attn_blocks(ic), fill)
    return proj_units(NCH - 1)


def build_program(reps=1):
    from contextlib import ExitStack

    nc = bacc.Bacc("TRN2", target_bir_lowering=False, debug=False)
    xT = nc.dram_tensor("xT", [C, T], BF16, kind="ExternalInput").ap()
    wq = nc.dram_tensor("wq", [C, OL], BF16, kind="ExternalInput").ap()
    wk = nc.dram_tensor("wk", [C, OL], BF16, kind="ExternalInput").ap()
    wv = nc.dram_tensor("wv", [C, OL], BF16, kind="ExternalInput").ap()
    wp = nc.dram_tensor("wp", [OL, C], BF16, kind="ExternalInput").ap()
    tri = nc.dram_tensor("tri", [128, 128], BF16, kind="ExternalInput").ap()
    tri2 = nc.dram_tensor("tri2", [128, 256], BF16, kind="ExternalInput").ap()
    onesd = nc.dram_tensor("onesd", [128, TT * HL * 64], BF16, kind="ExternalInput").ap()
    out = nc.dram_tensor("out", [T, C], BF16, kind="ExternalOutput").ap()

    with tile.TileContext(nc) as tc:
        with ExitStack() as ctx:
            pools = {
                "inp2": ctx.enter_context(tc.tile_pool(name="inp2", bufs=2)),
                "cst1": ctx.enter_context(tc.tile_pool(name="cst1", bufs=1)),
                "act2": ctx.enter_context(tc.tile_pool(name="act2", bufs=2)),
                "pp": ctx.enter_context(tc.tile_pool(name="pp", bufs=4)),
                "rcp": ctx.enter_context(tc.tile_pool(name="rcp", bufs=2)),
                "bbp": ctx.enter_context(tc.tile_pool(name="bbp", bufs=2)),
                "obp": ctx.enter_context(tc.tile_pool(name="obp", bufs=3)),
                "ps_mm": ctx.enter_context(tc.tile_pool(name="ps_mm", bufs=2, space="PSUM")),
                "ps_pj": ctx.enter_context(tc.tile_pool(name="ps_pj", bufs=2, space="PSUM")),
                "ps_ot": ctx.enter_context(tc.tile_pool(name="ps_ot", bufs=2, space="PSUM")),
            }
            shared = {}
            carry = []
            for rep in range(reps):
                carry = build_body(tc, pools, rep, shared, carry,
                                   xT, wq, wk, wv, wp, tri, tri2, onesd, out)
            for u in carry:
                u()
    nc.compile()
    return nc


def quant_weight_np(w):
    scale = max(np.mean(np.abs(w), dtype=np.float32), np.float32(1e-8))
    return (np.clip(np.round(w / scale), -2.0, 2.0) * scale).astype(np.float32)


def make_in_maps(x, w_attn, w_proj):
    wq_f = quant_weight_np(w_attn)
    wp_f = quant_weight_np(w_proj)
    onesd_v = np.zeros((128, TT * HL, 64), dtype=ml_dtypes.bfloat16)
    onesd_v[:, :, 0] = 1
    onesd_v = onesd_v.reshape(128, TT * HL * 64)
    tri = np.triu(np.ones((128, 128), dtype=np.float32))
    tri2 = np.concatenate([tri, np.ones((128, 128), dtype=np.float32)], axis=1)
    in_maps = []
    for core in range(8):
        b, g = divmod(core, 4)
        sl = slice(g * OL, (g + 1) * OL)
        in_maps.append({
            "xT": np.ascontiguousarray(x[b].T).astype(ml_dtypes.bfloat16),
            "wq": np.ascontiguousarray(wq_f[0 * C:1 * C][sl].T).astype(ml_dtypes.bfloat16),
            "wk": np.ascontiguousarray(wq_f[1 * C:2 * C][sl].T).astype(ml_dtypes.bfloat16),
            "wv": np.ascontiguousarray(wq_f[2 * C:3 * C][sl].T).astype(ml_dtypes.bfloat16),
            "wp": np.ascontiguousarray(wp_f[:, sl].T).astype(ml_dtypes.bfloat16),
            "tri": tri.astype(ml_dtypes.bfloat16),
            "tri2": tri2.astype(ml_dtypes.bfloat16),
            "onesd": onesd_v,
        })
    return in_maps


_CACHED_NC = None


def kernel(x, w_attn, w_proj):
    global _CACHED_NC
    if _CACHED_NC is None:
        _CACHED_NC = build_program()
    in_maps = make_in_maps(np.asarray(x, dtype=np.float32),
                           np.asarray(w_attn, dtype=np.float32),
                           np.asarray(w_proj, dtype=np.float32))
    res = run_bass_kernel_spmd(_CACHED_NC, in_maps, list(range(8)))
    out = np.zeros((B, T, C), dtype=np.float32)
    for core in range(8):
        b = core // 4
        out[b] += res.results[core]["out"]
    return out
